# revision 1
# baseline (speedup 1.0000x reference)
"""Trainium2 Bass kernel for the GNN message-passing problem (GAT+GAT+GCN+pool+MLP).

Self-contained: hardcodes problem shapes. Distributes across 8 NeuronCores by
sharding destination nodes (degree-sorted, round-robin balanced across cores);
gathers source-node rows per edge via indirect DMA from AllGathered projection
buffers; all floating-point model compute runs on device.

Layout: nodes are permuted so that core c owns new-ids [c*npc, (c+1)*npc),
tiled in groups of 128 (partition = destination node). Edge slots for each
dst node lie along the free dimension, padded per tile to a shared max degree
so the single SPMD program is identical across cores.
"""

import math
import sys

import numpy as np

sys.path.insert(0, "/opt/trn_rl_repo")

from concourse import bacc, mybir, tile  # noqa: E402
from concourse.bass import AP, IndirectOffsetOnAxis  # noqa: E402

P = 128
NCORES = 8
F32 = mybir.dt.float32
I32 = mybir.dt.int32
ALU = mybir.AluOpType
ACTF = mybir.ActivationFunctionType
AX = mybir.AxisListType

F = 128          # input features
H = 64           # hidden
NHEAD = 4

CAP1 = 28        # max gather chunk (edge slots) for GAT1 (264 f32 cols/row)
CAP2 = 56        # GAT2 (130 cols/row)
CAP3 = 112       # GCN (64 cols/row)


def _chunks(d, cap):
    n = (d + cap - 1) // cap
    base, rem = d // n, d % n
    out, j = [], 0
    for i in range(n):
        c = base + (1 if i < rem else 0)
        out.append((j, j + c))
        j += c
    return out


class Meta:
    def __init__(self, n_pad, tpc, d, b):
        self.n_pad = n_pad
        self.tpc = tpc
        self.d = d
        self.doff = [0]
        for v in d:
            self.doff.append(self.doff[-1] + v)
        self.sumd = self.doff[-1]
        self.b = b
        self.gchunk = (b + P - 1) // P
        self.c1 = 4 * H + 2 * NHEAD   # 264
        self.c2 = 2 * H + 2           # 130
        self.c3 = H                   # 64
        self.ncls = 10

    def key(self):
        return (self.n_pad, self.tpc, self.b, tuple(self.d))


def preprocess(x, edge_index, batch, W1, a_src1, a_dst1, b1, W2, a_src2, a_dst2, b2,
               Wg, bg, bn1_g, bn1_b, bn2_g, bn2_b, bn3_g, bn3_b,
               lin1_W, lin1_b, lin2_W, lin2_b, batch_size):
    """Host-side index/partition preprocessing. Returns (meta, per-core input dicts)."""
    x = np.asarray(x, np.float32)
    n = x.shape[0]
    bsz = int(batch_size)
    ei = np.asarray(edge_index, np.int64)
    src = np.concatenate([ei[0], np.arange(n, dtype=np.int64)])
    dst = np.concatenate([ei[1], np.arange(n, dtype=np.int64)])
    deg = np.bincount(dst, minlength=n).astype(np.int64)

    tpc = math.ceil((n + 1) / (NCORES * P))   # +1 guarantees >= 1 phantom node
    n_pad = NCORES * P * tpc
    npc = tpc * P
    deg_pad = np.concatenate([deg, np.zeros(n_pad - n, np.int64)])

    order = np.argsort(-deg_pad, kind="stable")        # rank -> old id
    r = np.arange(n_pad)
    g = r // P
    newid_of_rank = (g % NCORES) * npc + (g // NCORES) * P + (r % P)
    new_of_old = np.empty(n_pad, np.int64)
    new_of_old[order] = newid_of_rank
    old_of_new = np.empty(n_pad, np.int64)
    old_of_new[newid_of_rank] = order

    d_tile = deg_pad[order].reshape(tpc, NCORES * P).max(1)
    d_tile = np.maximum(d_tile, 1).astype(np.int64)

    pad_id = n_pad - 1
    assert deg_pad[old_of_new[pad_id]] == 0, "need at least one phantom node"

    meta = Meta(n_pad=n_pad, tpc=tpc, d=[int(v) for v in d_tile], b=bsz)
    doff = meta.doff
    sumd = meta.sumd

    # CSR over new dst ids
    dstn = new_of_old[dst]
    srcn = new_of_old[src]
    eorder = np.argsort(dstn, kind="stable")
    srcs = srcn[eorder].astype(np.int64)
    cnt = np.bincount(dstn, minlength=n_pad)
    rowptr = np.concatenate([[0], np.cumsum(cnt)]).astype(np.int64)

    offs = np.full((NCORES, P, sumd), pad_id, np.int32)
    gmask = np.zeros((NCORES, P, sumd), np.float32)
    for c in range(NCORES):
        cbase = c * npc
        for t in range(tpc):
            base_new = cbase + t * P
            o = doff[t]
            for p in range(P):
                a, bnd = rowptr[base_new + p], rowptr[base_new + p + 1]
                k = bnd - a
                if k:
                    offs[c, p, o:o + k] = srcs[a:bnd]
                    gmask[c, p, o:o + k] = 1.0

    deg_new = deg_pad[old_of_new]
    dis_new = np.where(deg_new > 0, 1.0 / np.sqrt(np.maximum(deg_new, 1)), 0.0).astype(np.float32)
    batch_new = np.full(n_pad, -1.0, np.float32)
    real = old_of_new < n
    batch_new[real] = np.asarray(batch, np.float64)[old_of_new[real]]

    x_new = np.zeros((n_pad, F), np.float32)
    x_new[real] = x[old_of_new[real]]

    # weights
    W1 = np.asarray(W1, np.float32)
    a_src1 = np.asarray(a_src1, np.float32)
    a_dst1 = np.asarray(a_dst1, np.float32)
    ws1 = np.stack([W1[:, h * H:(h + 1) * H] @ a_src1[h] for h in range(NHEAD)], 1)
    wd1 = np.stack([W1[:, h * H:(h + 1) * H] @ a_dst1[h] for h in range(NHEAD)], 1)
    w1ext = np.ascontiguousarray(np.concatenate([W1, ws1, wd1], 1))        # [128, 264]
    W2 = np.asarray(W2, np.float32)
    ws2 = W2 @ np.asarray(a_src2, np.float32)[0]
    wd2 = W2 @ np.asarray(a_dst2, np.float32)[0]
    w2ext = np.ascontiguousarray(np.concatenate([W2, ws2[:, None], wd2[:, None]], 1))  # [256, 130]

    eps = 1e-5
    s1 = np.asarray(bn1_g, np.float32) / np.float32(np.sqrt(1.0 + eps))
    s2 = np.asarray(bn2_g, np.float32) / np.float32(np.sqrt(1.0 + eps))
    s3 = np.asarray(bn3_g, np.float32) / np.float32(np.sqrt(1.0 + eps))
    l1w = np.ascontiguousarray(np.asarray(lin1_W, np.float32) * s3[None, :])
    l1b = np.asarray(lin1_b, np.float32) * s3 + np.asarray(bn3_b, np.float32)

    def rep(v):
        v = np.asarray(v, np.float32)
        return np.ascontiguousarray(np.broadcast_to(v[None, :], (P, v.shape[0])))

    gcnt = np.bincount(np.asarray(batch, np.int64), minlength=bsz).astype(np.float32)
    b_pad = meta.gchunk * P
    inv_pad = np.zeros(b_pad, np.float32)
    inv_pad[:bsz] = 1.0 / np.maximum(gcnt, 1.0)
    invc = np.ascontiguousarray(inv_pad.reshape(meta.gchunk, P).T)          # [128, gchunk]
    iota = np.ascontiguousarray(
        np.broadcast_to(np.arange(b_pad, dtype=np.float32)[None, :], (P, b_pad)))

    in_maps = []
    for c in range(NCORES):
        sl = slice(c * npc, (c + 1) * npc)
        in_maps.append({
            "xT": np.ascontiguousarray(x_new[sl].T),
            "offs": np.ascontiguousarray(offs[c]),
            "gmask": np.ascontiguousarray(gmask[c]),
            "tdis": np.ascontiguousarray(dis_new[sl].reshape(tpc, P).T),
            "tbatch": np.ascontiguousarray(batch_new[sl].reshape(tpc, P).T),
            "w1ext": w1ext,
            "w2a": np.ascontiguousarray(w2ext[:F]),
            "w2b": np.ascontiguousarray(w2ext[F:]),
            "wg": np.ascontiguousarray(np.asarray(Wg, np.float32)),
            "cb1": rep(b1), "cs1": rep(s1), "ct1": rep(bn1_b),
            "cb2": rep(b2), "cs2": rep(s2), "ct2": rep(bn2_b),
            "cbg": rep(bg),
            "l1w": l1w, "l1b": rep(l1b),
            "l2w": np.ascontiguousarray(np.asarray(lin2_W, np.float32)), "l2b": rep(lin2_b),
            "invc": invc, "iota": iota,
            "identin": np.ascontiguousarray(np.eye(P, dtype=np.float32)),
        })
    return meta, in_maps


def _ap(base: AP, dims):
    """Custom AP on base's tensor: partition pair from base + given [step, count] dims."""
    return AP(base.tensor, base.offset, [list(base.ap[0])] + [list(p) for p in dims])


def build(meta: Meta):
    m = meta
    npc = m.tpc * P
    nc = bacc.Bacc(None, target_bir_lowering=False)

    def di(name, shape, dt=F32):
        return nc.dram_tensor(name, shape, dt, kind="ExternalInput")

    xT = di("xT", [P, npc])
    offs = di("offs", [P, m.sumd], I32)
    gmask = di("gmask", [P, m.sumd])
    tdis = di("tdis", [P, m.tpc])
    tbatch = di("tbatch", [P, m.tpc])
    w1ext = di("w1ext", [F, m.c1])
    w2a = di("w2a", [F, m.c2])
    w2b = di("w2b", [F, m.c2])
    wg = di("wg", [2 * H, H])
    cb1 = di("cb1", [P, 4 * H]); cs1 = di("cs1", [P, 4 * H]); ct1 = di("ct1", [P, 4 * H])
    cb2 = di("cb2", [P, 2 * H]); cs2 = di("cs2", [P, 2 * H]); ct2 = di("ct2", [P, 2 * H])
    cbg = di("cbg", [P, H])
    l1w = di("l1w", [H, F]); l1b = di("l1b", [P, F])
    l2w = di("l2w", [F, m.ncls]); l2b = di("l2b", [P, m.ncls])
    invc = di("invc", [P, m.gchunk])
    iota = di("iota", [P, m.gchunk * P])
    identin = di("identin", [P, P])
    out = nc.dram_tensor("out", [m.b, m.ncls], F32, kind="ExternalOutput")

    h1sh = nc.dram_tensor("h1sh", [npc, m.c1], F32)
    h1full = nc.dram_tensor("h1full", [m.n_pad, m.c1], F32, addr_space="Shared")
    h2sh = nc.dram_tensor("h2sh", [npc, m.c2], F32)
    h2full = nc.dram_tensor("h2full", [m.n_pad, m.c2], F32, addr_space="Shared")
    h3sh = nc.dram_tensor("h3sh", [npc, m.c3], F32)
    h3full = nc.dram_tensor("h3full", [m.n_pad, m.c3], F32, addr_space="Shared")
    psh = nc.dram_tensor("psh", [P, m.gchunk * H], F32)
    pfull = nc.dram_tensor("pfull", [P, m.gchunk * H], F32, addr_space="Shared")

    RG = [list(range(NCORES))]

    with tile.TileContext(nc) as tc:
        with (
            tc.tile_pool(name="res", bufs=1) as res,
            tc.tile_pool(name="work", bufs=2) as wk,
            tc.tile_pool(name="hgp", bufs=2) as hgp,
            tc.tile_pool(name="psum", bufs=3, space="PSUM") as pp,
            tc.tile_pool(name="pacc", bufs=1, space="PSUM") as pacc,
        ):
            def ld(dram, dt=F32, tag=None):
                t_ = res.tile(list(dram.shape), dtype=dt, tag=tag or dram.name)
                nc.sync.dma_start(out=t_[:], in_=dram[:])
                return t_

            xT_s = ld(xT)
            offs_s = ld(offs, dt=I32)
            gmask_s = ld(gmask)
            tdis_s = ld(tdis)
            tbatch_s = ld(tbatch)
            w1_s = ld(w1ext)
            w2a_s = ld(w2a)
            w2b_s = ld(w2b)
            wg_s = ld(wg)
            cb1_s = ld(cb1); cs1_s = ld(cs1); ct1_s = ld(ct1)
            cb2_s = ld(cb2); cs2_s = ld(cs2); ct2_s = ld(ct2)
            cbg_s = ld(cbg)
            l1w_s = ld(l1w); l1b_s = ld(l1b)
            l2w_s = ld(l2w); l2b_s = ld(l2b)
            invc_s = ld(invc)
            iota_s = ld(iota)
            ident = ld(identin, tag="ident")

            x1T_s = res.tile([P, 2 * npc], dtype=F32, tag="x1T")
            x2T_s = res.tile([P, npc], dtype=F32, tag="x2T")
            scd1_s = res.tile([P, m.tpc * NHEAD], dtype=F32, tag="scd1")
            scd2_s = res.tile([P, m.tpc], dtype=F32, tag="scd2")

            def lrelu(ap_io, alpha, tmp_shape, tag):
                """in-place leaky relu: v = max(v, alpha*v) via ACT mul + DVE max."""
                tmp = wk.tile(tmp_shape, dtype=F32, tag=tag)
                ta = tmp[:]
                if len(tmp_shape) == 3:
                    ta = tmp[:, :ap_io.shape[1], :]
                elif tmp_shape[1] != ap_io.shape[1]:
                    ta = tmp[:, :ap_io.shape[1]]
                nc.scalar.activation(ta, ap_io, ACTF.Copy, scale=float(alpha))
                nc.vector.tensor_tensor(out=ap_io, in0=ap_io, in1=ta, op=ALU.max)

            # =========== proj1: h1ext = xT.T @ w1ext ===========
            for t in range(m.tpc):
                ps = pp.tile([P, m.c1], dtype=F32, tag="ps")
                nc.tensor.matmul(ps[:], lhsT=xT_s[:, t * P:(t + 1) * P], rhs=w1_s[:],
                                 start=True, stop=True)
                hb = wk.tile([P, m.c1], dtype=F32, tag="projsb")
                nc.vector.tensor_copy(hb[:], ps[:])
                nc.vector.tensor_copy(scd1_s[:, t * NHEAD:(t + 1) * NHEAD],
                                      hb[:, 4 * H + NHEAD:4 * H + 2 * NHEAD])
                nc.sync.dma_start(out=h1sh[t * P:(t + 1) * P, :], in_=hb[:])

            nc.gpsimd.collective_compute(
                "AllGather", ALU.bypass, replica_groups=RG,
                ins=[h1sh.ap().opt()], outs=[h1full.ap().opt()])

            # =========== GAT1 aggregation ===========
            for t in range(m.tpc):
                agg = wk.tile([P, 4 * H], dtype=F32, tag="agg1")
                ssum = wk.tile([P, NHEAD], dtype=F32, tag="ssum1")
                first = True
                for (j0, j1) in _chunks(m.d[t], CAP1):
                    dc = j1 - j0
                    hg = hgp.tile([P, CAP1, m.c1], dtype=F32, tag="hg")
                    for j in range(j0, j1):
                        nc.gpsimd.indirect_dma_start(
                            out=hg[:, j - j0, :], out_offset=None, in_=h1full[:, :],
                            in_offset=IndirectOffsetOnAxis(
                                ap=offs_s[:, m.doff[t] + j:m.doff[t] + j + 1], axis=0))
                    et = wk.tile([P, CAP1, NHEAD], dtype=F32, tag="et1")
                    nc.vector.tensor_tensor(
                        out=et[:, :dc, :], in0=hg[:, :dc, 4 * H:4 * H + NHEAD],
                        in1=_ap(scd1_s[:, t * NHEAD:(t + 1) * NHEAD], [[0, dc], [1, NHEAD]]),
                        op=ALU.add)
                    lrelu(et[:, :dc, :], 0.2, [P, CAP1, NHEAD], "lr1")
                    nc.scalar.activation(et[:, :dc, :], et[:, :dc, :], ACTF.Exp)
                    nc.vector.tensor_tensor(
                        out=et[:, :dc, :], in0=et[:, :dc, :],
                        in1=_ap(gmask_s[:, m.doff[t] + j0:m.doff[t] + j1], [[1, dc], [0, NHEAD]]),
                        op=ALU.mult)
                    et_hj = _ap(et[:], [[1, NHEAD], [NHEAD, dc]])
                    if first:
                        nc.vector.tensor_reduce(out=ssum[:], in_=et_hj, axis=AX.X, op=ALU.add)
                    else:
                        st = wk.tile([P, NHEAD], dtype=F32, tag="st1")
                        nc.vector.tensor_reduce(out=st[:], in_=et_hj, axis=AX.X, op=ALU.add)
                        nc.vector.tensor_add(out=ssum[:], in0=ssum[:], in1=st[:])
                    hg_jhf = _ap(hg[:], [[m.c1, dc], [H, NHEAD], [1, H]])
                    w_jhf = _ap(et[:], [[NHEAD, dc], [1, NHEAD], [0, H]])
                    nc.vector.tensor_tensor(out=hg_jhf, in0=hg_jhf, in1=w_jhf, op=ALU.mult)
                    hg_hfj = _ap(hg[:], [[H, NHEAD], [1, H], [m.c1, dc]])
                    if first:
                        nc.vector.tensor_reduce(out=agg[:], in_=hg_hfj, axis=AX.X, op=ALU.add)
                    else:
                        at = wk.tile([P, 4 * H], dtype=F32, tag="at1")
                        nc.vector.tensor_reduce(out=at[:], in_=hg_hfj, axis=AX.X, op=ALU.add)
                        nc.vector.tensor_add(out=agg[:], in0=agg[:], in1=at[:])
                    first = False
                nc.vector.tensor_scalar_add(out=ssum[:], in0=ssum[:], scalar1=1e-16)
                rs = wk.tile([P, NHEAD], dtype=F32, tag="rs1")
                nc.vector.reciprocal(rs[:], ssum[:])
                agg_hf = _ap(agg[:], [[H, NHEAD], [1, H]])
                nc.vector.tensor_tensor(out=agg_hf, in0=agg_hf,
                                        in1=_ap(rs[:], [[1, NHEAD], [0, H]]), op=ALU.mult)
                nc.vector.tensor_add(out=agg[:], in0=agg[:], in1=cb1_s[:])
                lrelu(agg[:], 0.01, [P, 4 * H], "lre1")
                x1 = wk.tile([P, 4 * H], dtype=F32, tag="x1")
                nc.vector.tensor_mul(out=x1[:], in0=agg[:], in1=cs1_s[:])
                nc.vector.tensor_add(out=x1[:], in0=x1[:], in1=ct1_s[:])
                for hf in range(2):
                    tp = pp.tile([P, P], dtype=F32, tag="ps")
                    nc.tensor.transpose(tp[:], x1[:, hf * P:(hf + 1) * P], ident[:])
                    nc.vector.tensor_copy(
                        x1T_s[:, hf * npc + t * P:hf * npc + (t + 1) * P], tp[:])

            # =========== proj2 ===========
            for t in range(m.tpc):
                ps = pp.tile([P, m.c2], dtype=F32, tag="ps")
                nc.tensor.matmul(ps[:], lhsT=x1T_s[:, t * P:(t + 1) * P],
                                 rhs=w2a_s[:], start=True, stop=False)
                nc.tensor.matmul(ps[:], lhsT=x1T_s[:, npc + t * P:npc + (t + 1) * P],
                                 rhs=w2b_s[:], start=False, stop=True)
                hb = wk.tile([P, m.c2], dtype=F32, tag="projsb")
                nc.vector.tensor_copy(hb[:], ps[:])
                nc.vector.tensor_copy(scd2_s[:, t:t + 1], hb[:, 2 * H + 1:2 * H + 2])
                nc.sync.dma_start(out=h2sh[t * P:(t + 1) * P, :], in_=hb[:])

            nc.gpsimd.collective_compute(
                "AllGather", ALU.bypass, replica_groups=RG,
                ins=[h2sh.ap().opt()], outs=[h2full.ap().opt()])

            # =========== GAT2 aggregation ===========
            for t in range(m.tpc):
                agg = wk.tile([P, 2 * H], dtype=F32, tag="agg2")
                ssum = wk.tile([P, 1], dtype=F32, tag="ssum2")
                first = True
                for (j0, j1) in _chunks(m.d[t], CAP2):
                    dc = j1 - j0
                    hg = hgp.tile([P, CAP2, m.c2], dtype=F32, tag="hg")
                    for j in range(j0, j1):
                        nc.gpsimd.indirect_dma_start(
                            out=hg[:, j - j0, :], out_offset=None, in_=h2full[:, :],
                            in_offset=IndirectOffsetOnAxis(
                                ap=offs_s[:, m.doff[t] + j:m.doff[t] + j + 1], axis=0))
                    et = wk.tile([P, CAP2], dtype=F32, tag="et2")
                    nc.vector.tensor_scalar_add(out=et[:, :dc], in0=hg[:, :dc, 2 * H],
                                                scalar1=scd2_s[:, t:t + 1])
                    lrelu(et[:, :dc], 0.2, [P, CAP2], "lr2")
                    nc.scalar.activation(et[:, :dc], et[:, :dc], ACTF.Exp)
                    nc.vector.tensor_tensor(
                        out=et[:, :dc], in0=et[:, :dc],
                        in1=gmask_s[:, m.doff[t] + j0:m.doff[t] + j1], op=ALU.mult)
                    if first:
                        nc.vector.tensor_reduce(out=ssum[:], in_=et[:, :dc], axis=AX.X, op=ALU.add)
                    else:
                        st = wk.tile([P, 1], dtype=F32, tag="st2")
                        nc.vector.tensor_reduce(out=st[:], in_=et[:, :dc], axis=AX.X, op=ALU.add)
                        nc.vector.tensor_add(out=ssum[:], in0=ssum[:], in1=st[:])
                    hg_jf = _ap(hg[:], [[m.c2, dc], [1, 2 * H]])
                    w_jf = _ap(et[:], [[1, dc], [0, 2 * H]])
                    nc.vector.tensor_tensor(out=hg_jf, in0=hg_jf, in1=w_jf, op=ALU.mult)
                    hg_fj = _ap(hg[:], [[1, 2 * H], [m.c2, dc]])
                    if first:
                        nc.vector.tensor_reduce(out=agg[:], in_=hg_fj, axis=AX.X, op=ALU.add)
                    else:
                        at = wk.tile([P, 2 * H], dtype=F32, tag="at2")
                        nc.vector.tensor_reduce(out=at[:], in_=hg_fj, axis=AX.X, op=ALU.add)
                        nc.vector.tensor_add(out=agg[:], in0=agg[:], in1=at[:])
                    first = False
                nc.vector.tensor_scalar_add(out=ssum[:], in0=ssum[:], scalar1=1e-16)
                rs = wk.tile([P, 1], dtype=F32, tag="rs2")
                nc.vector.reciprocal(rs[:], ssum[:])
                nc.vector.tensor_tensor(out=agg[:], in0=agg[:],
                                        in1=_ap(rs[:], [[0, 2 * H]]), op=ALU.mult)
                nc.vector.tensor_add(out=agg[:], in0=agg[:], in1=cb2_s[:])
                lrelu(agg[:], 0.01, [P, 2 * H], "lre2")
                x2 = wk.tile([P, 2 * H], dtype=F32, tag="x2")
                nc.vector.tensor_mul(out=x2[:], in0=agg[:], in1=cs2_s[:])
                nc.vector.tensor_add(out=x2[:], in0=x2[:], in1=ct2_s[:])
                tp = pp.tile([P, P], dtype=F32, tag="ps")
                nc.tensor.transpose(tp[:], x2[:], ident[:])
                nc.vector.tensor_copy(x2T_s[:, t * P:(t + 1) * P], tp[:])

            # =========== proj3: h3 = (x2 @ Wg) * dis[src] ===========
            for t in range(m.tpc):
                ps = pp.tile([P, H], dtype=F32, tag="ps")
                nc.tensor.matmul(ps[:], lhsT=x2T_s[:, t * P:(t + 1) * P], rhs=wg_s[:],
                                 start=True, stop=True)
                hb = wk.tile([P, H], dtype=F32, tag="projsb")
                nc.vector.tensor_scalar_mul(out=hb[:], in0=ps[:], scalar1=tdis_s[:, t:t + 1])
                nc.sync.dma_start(out=h3sh[t * P:(t + 1) * P, :], in_=hb[:])

            nc.gpsimd.collective_compute(
                "AllGather", ALU.bypass, replica_groups=RG,
                ins=[h3sh.ap().opt()], outs=[h3full.ap().opt()])

            # =========== GCN aggregation + pooling ===========
            poolps = []
            for gc in range(m.gchunk):
                pooltile = pacc.tile([P, H], dtype=F32, tag=f"poolps{gc}", name=f"poolps{gc}")
                poolps.append(pooltile)
            for t in range(m.tpc):
                agg = wk.tile([P, H], dtype=F32, tag="agg3")
                first = True
                for (j0, j1) in _chunks(m.d[t], CAP3):
                    dc = j1 - j0
                    hg = hgp.tile([P, CAP3, m.c3], dtype=F32, tag="hg")
                    for j in range(j0, j1):
                        nc.gpsimd.indirect_dma_start(
                            out=hg[:, j - j0, :], out_offset=None, in_=h3full[:, :],
                            in_offset=IndirectOffsetOnAxis(
                                ap=offs_s[:, m.doff[t] + j:m.doff[t] + j + 1], axis=0))
                    hg_fj = _ap(hg[:], [[1, H], [m.c3, dc]])
                    if first:
                        nc.vector.tensor_reduce(out=agg[:], in_=hg_fj, axis=AX.X, op=ALU.add)
                    else:
                        at = wk.tile([P, H], dtype=F32, tag="at3")
                        nc.vector.tensor_reduce(out=at[:], in_=hg_fj, axis=AX.X, op=ALU.add)
                        nc.vector.tensor_add(out=agg[:], in0=agg[:], in1=at[:])
                    first = False
                x3 = wk.tile([P, H], dtype=F32, tag="x3")
                nc.vector.tensor_scalar_mul(out=x3[:], in0=agg[:], scalar1=tdis_s[:, t:t + 1])
                nc.vector.tensor_add(out=x3[:], in0=x3[:], in1=cbg_s[:])
                lrelu(x3[:], 0.01, [P, H], "lre3")
                for gc in range(m.gchunk):
                    sel = wk.tile([P, P], dtype=F32, tag="sel")
                    nc.vector.tensor_scalar(
                        out=sel[:], in0=iota_s[:, gc * P:(gc + 1) * P],
                        scalar1=tbatch_s[:, t:t + 1], scalar2=None, op0=ALU.is_equal)
                    nc.tensor.matmul(poolps[gc][:], lhsT=sel[:], rhs=x3[:],
                                     start=(t == 0), stop=(t == m.tpc - 1))

            # =========== pooled AllReduce + MLP ===========
            psb = wk.tile([P, m.gchunk * H], dtype=F32, tag="psb")
            for gc in range(m.gchunk):
                nc.vector.tensor_copy(psb[:, gc * H:(gc + 1) * H], poolps[gc][:])
            nc.sync.dma_start(out=psh[:, :], in_=psb[:])
            nc.gpsimd.collective_compute(
                "AllReduce", ALU.add, replica_groups=RG,
                ins=[psh.ap().opt()], outs=[pfull.ap().opt()])
            pf = wk.tile([P, m.gchunk * H], dtype=F32, tag="pf")
            nc.sync.dma_start(out=pf[:], in_=pfull[:, :])
            pf_gf = _ap(pf[:], [[H, m.gchunk], [1, H]])
            nc.vector.tensor_tensor(out=pf_gf, in0=pf_gf,
                                    in1=_ap(invc_s[:], [[1, m.gchunk], [0, H]]), op=ALU.mult)
            pT = wk.tile([H, m.gchunk * P], dtype=F32, tag="pT")
            for gc in range(m.gchunk):
                tp = pp.tile([P, P], dtype=F32, tag="ps")
                nc.tensor.transpose(tp[:H, :], pf[:, gc * H:(gc + 1) * H], ident[:])
                nc.vector.tensor_copy(pT[:, gc * P:(gc + 1) * P], tp[:H, :])
            yT = wk.tile([P, m.gchunk * P], dtype=F32, tag="yT")
            for gc in range(m.gchunk):
                yps = pp.tile([P, F], dtype=F32, tag="ps")
                nc.tensor.matmul(yps[:], lhsT=pT[:, gc * P:(gc + 1) * P], rhs=l1w_s[:],
                                 start=True, stop=True)
                ysb = wk.tile([P, F], dtype=F32, tag="ysb")
                nc.vector.tensor_add(out=ysb[:], in0=yps[:], in1=l1b_s[:])
                lrelu(ysb[:], 0.01, [P, F], "lrey")
                tp = pp.tile([P, P], dtype=F32, tag="ps")
                nc.tensor.transpose(tp[:], ysb[:], ident[:])
                nc.vector.tensor_copy(yT[:, gc * P:(gc + 1) * P], tp[:])
            for gc in range(m.gchunk):
                ops_ = pp.tile([P, m.ncls], dtype=F32, tag="ps")
                nc.tensor.matmul(ops_[:], lhsT=yT[:, gc * P:(gc + 1) * P], rhs=l2w_s[:],
                                 start=True, stop=True)
                osb = wk.tile([P, m.ncls], dtype=F32, tag="osb")
                nc.vector.tensor_add(out=osb[:], in0=ops_[:], in1=l2b_s[:])
                r0, r1 = gc * P, min((gc + 1) * P, m.b)
                if r1 > r0:
                    nc.sync.dma_start(out=out[r0:r1, :], in_=osb[:r1 - r0, :])

    nc.compile()
    return nc


_CACHE = {}


def kernel(**inputs):
    meta, in_maps = preprocess(**inputs)
    key = meta.key()
    if key not in _CACHE:
        _CACHE[key] = build(meta)
    nc = _CACHE[key]
    from concourse import bass_utils
    res = bass_utils.run_bass_kernel_spmd(nc, in_maps, core_ids=list(range(NCORES)))
    return np.asarray(res.results[0]["out"])

